# revision 29
# baseline (speedup 1.0000x reference)
"""Trainium2 Bass kernel for nn_MetaController (GRU + gated scan + hypernet decoder).

Self-contained: kernel(**inputs) -> np.ndarray [2,1024,1024] float32.

Two SPMD programs on 8 NeuronCores:
  P1: GRU solved by quasi-DEER fixed-point iteration (diagonal-Jacobian
      Newton): init with the W_hh=0 diagonal model (one tensor_tensor_scan),
      then NSWEEP sweeps of [all-step gate matmul (tensor-parallel over
      hidden channels, 8 cores) -> gate nonlinearities -> diagonal linear
      recurrence via tensor_tensor_scan]. Each sweep all-gathers the updated
      h trajectory (bf16) via remote SBUF DMA broadcast with parity
      double-buffering. Emits per-core partial beta projections; host
      applies sigmoid. Converges ~4.3x/sweep; NSWEEP=2 => beta err ~1.2e-3,
      comparable to the bf16 noise floor (total rel err 5.3e-3 vs 2e-2 gate).
  P2: gated associative scan via DVE tensor_tensor_scan, decoder mm1 (native
      Gelu_apprx_tanh) replicated, 16384-row w1-half of the decoder output
      tensor-parallel in r-major row order so the low-rank contraction
      sum_r w1*(w2 row-sums) becomes 16 broadcast-multiply-accumulates.
      The w2-half collapses to 16 columns via host-presummed W2s.
"""
import sys
sys.path.insert(0, '/opt/trn_rl_repo')
import numpy as np
import ml_dtypes
import concourse.bass as bass
import concourse.mybir as mybir
from concourse.bass import ds
from concourse import library_config, library_overlay, bacc
from concourse.tile import TileContext
from concourse.bass_utils import run_bass_kernel_spmd

F32 = mybir.dt.float32
BF16 = mybir.dt.bfloat16
I32 = mybir.dt.int32
AF = mybir.ActivationFunctionType
ALU = mybir.AluOpType

B, N, D, R, H = 2, 1024, 1024, 16, 2048
P = 128
BT = B * N          # 2048 tokens, b-major
NSWEEP = 2


# ------------------------------------------------------------------ P1 (GRU)
def _p1_host_prep(inputs, core):
    lat = np.asarray(inputs["latent"], np.float32)
    w_ih = np.asarray(inputs["gru_w_ih"], np.float32)
    w_hh = np.asarray(inputs["gru_w_hh"], np.float32)
    assert not np.any(np.asarray(inputs["gru_b_ih"])), "nonzero b_ih unsupported"
    assert not np.any(np.asarray(inputs["gru_b_hh"])), "nonzero b_hh unsupported"
    beta_w = np.asarray(inputs["beta_w"], np.float32)
    c = core
    bf = ml_dtypes.bfloat16
    sl = slice(c * P, (c + 1) * P)

    latT = np.ascontiguousarray(lat.transpose(2, 0, 1).reshape(D, BT))
    # lhsT layouts [D, 3*P]: col g*P+j = gate g, own channel j
    wih = w_ih.reshape(3, D, D)[:, sl, :]          # [3, P, D]
    whh = w_hh.reshape(3, D, D)[:, sl, :]
    wih_lhsT = np.ascontiguousarray(wih.transpose(2, 0, 1).reshape(D, 3 * P))
    whh_lhsT = np.ascontiguousarray(whh.transpose(2, 0, 1).reshape(D, 3 * P))
    slot4 = np.array([[c * BT, c * BT + N,
                       8 * BT + c * BT, 8 * BT + c * BT + N]], np.int32)
    return {
        "latT_tb": latT.astype(bf),
        "wih_lhsT": wih_lhsT.astype(bf),
        "whh_lhsT": whh_lhsT.astype(bf),
        "bw_pc": np.ascontiguousarray(beta_w[0, sl][:, None]).astype(bf),
        "slot4": slot4,
    }


def _p1_build(nc):
    latT_tb = nc.declare_dram_parameter("latT_tb", [D, BT], BF16, isOutput=False)
    wih_l = nc.declare_dram_parameter("wih_lhsT", [D, 3 * P], BF16, isOutput=False)
    whh_l = nc.declare_dram_parameter("whh_lhsT", [D, 3 * P], BF16, isOutput=False)
    bw_pc = nc.declare_dram_parameter("bw_pc", [P, 1], BF16, isOutput=False)
    slot4 = nc.declare_dram_parameter("slot4", [1, 4], I32, isOutput=False)
    betap = nc.declare_dram_parameter("betap", [1, BT], F32, isOutput=True)

    from contextlib import ExitStack
    with ExitStack() as ctx:
        def sbuf(name, shape, dtype):
            return ctx.enter_context(nc.sbuf_tensor(name, shape, dtype))

        def sem(name):
            return ctx.enter_context(nc.semaphore(name))

        lat_ch = sbuf("lat_ch", [P, 8 * BT], BF16)          # all 8 chunks resident
        wih_s = sbuf("wih_s", [P, 8 * 3 * P], BF16)         # k-major chunks
        whh_s = sbuf("whh_s", [P, 8 * 3 * P], BF16)
        xp_s = sbuf("xp_s", [P, 3 * BT], F32)               # per-gate xp
        Hsh = sbuf("Hsh", [P, 2 * 8 * BT], BF16)            # parity,chunk,b,t+1
        # col 0 = zero, cols 1..1024 = b0 h, col 1040 = zero, 1041..2064 = b1 h
        Hbf = sbuf("Hbf", [P, 2080], BF16)
        rz_s = sbuf("rz_s", [P, 2 * BT], F32)               # sigmoid out r,z
        ta = sbuf("ta", [P, BT], F32)
        tb = sbuf("tb", [P, BT], F32)
        tcn = sbuf("tcn", [P, BT], F32)                     # n = tanh(...)
        betap_s = sbuf("betap_s", [1, BT], F32)
        bw_s = sbuf("bw_s", [P, 1], BF16)
        slot_s = sbuf("slot_s", [1, 4], I32)

        pa_r = ctx.enter_context(nc.psum_tensor("pa_r", [P, BT], F32))
        pa_z = ctx.enter_context(nc.psum_tensor("pa_z", [P, BT], F32))

        dma_sem = sem("dma_sem")
        s_w = sem("s_w")
        s_slot = sem("s_slot")
        s_xpk = sem("s_xpk")    # tensor: per lat-chunk MM group (16 total)
        s_xpc = sem("s_xpc")    # ACT xp drains (3)
        s_mm = sem("s_mm")      # tensor sweep groups: 2 per sweep
        s_pre = sem("s_pre")    # ACT xp->psum preloads: 2 per sweep
        s_sig = sem("s_sig")    # ACT sigmoid: init->1, sweep s->s+1
        s_tanh = sem("s_tanh")  # ACT tanh: init->1, sweep s->s+1
        s_tn = sem("s_tn")      # DVE tn (r*hn): sweep s->s
        s_tn2 = sem("s_tn2")    # DVE tn2: sweep s->s
        s_h = sem("s_h")        # b0 tts done: init->1, sweep s->s+1
        s_h1 = sem("s_h1")      # b1 tts done: same counting
        s_pb = sem("s_pb")
        s_ab = sem("s_ab")
        s_prep = sem("s_prep")
        rsem = sem("rsem")
        rsem2 = sem("rsem2")
        lsem = sem("lsem")

        def hsh_base(s, k):
            return ((s - 1) % 2) * 8 * BT + k * BT

        with nc.Block() as block:
            @block.sync
            def _(sync):
                sync.dma_start(out=wih_s[:].rearrange("p (k m) -> p k m", k=8),
                               in_=wih_l[:, :].rearrange("(k p) m -> p k m", p=P)
                               ).then_inc(s_w, 16)
                sync.dma_start(out=whh_s[:].rearrange("p (k m) -> p k m", k=8),
                               in_=whh_l[:, :].rearrange("(k p) m -> p k m", p=P)
                               ).then_inc(s_w, 16)
                sync.dma_start(out=bw_s[:], in_=bw_pc[:, :]).then_inc(s_w, 16)
                sync.dma_start(out=slot_s[:], in_=slot4[:, :]).then_inc(s_slot, 16)
                # latent: one DMA, loaded once; both xp passes read from SBUF
                sync.dma_start(out=lat_ch[:].rearrange("p (k c) -> p k c", k=8),
                               in_=latT_tb[:, :].rearrange("(k p) c -> p k c", p=P)
                               ).then_inc(dma_sem, 16)
                sync.wait_ge(s_ab, 1)
                sync.dma_start(out=betap[:, :], in_=betap_s[:]).then_inc(dma_sem, 16)
                sync.wait_ge(dma_sem, 32)
                sync.wait_ge(s_w, 48)

            @block.tensor
            def _(tensor):
                tensor.wait_ge(s_w, 48)
                tensor.wait_ge(dma_sem, 16)           # latent resident
                # xp pass 1: gates r,z
                for i in range(8):
                    par = i
                    for g in range(2):
                        for jj in range(4):
                            mm = tensor.matmul(
                                (pa_r if g == 0 else pa_z)[:, jj * 512:(jj + 1) * 512],
                                wih_s[:, (i % 8) * 3 * P + g * P:(i % 8) * 3 * P + (g + 1) * P],
                                lat_ch[:, par * BT + jj * 512:par * BT + (jj + 1) * 512],
                                start=(i == 0), stop=(i == 7))
                    mm.then_inc(s_xpk, 1)
                # xp pass 2: gate n (into pa_r after ACT drained r)
                for i in range(8, 16):
                    if i == 8:
                        tensor.wait_ge(s_xpc, 1)
                    par = i % 8
                    for jj in range(4):
                        mm = tensor.matmul(
                            pa_r[:, jj * 512:(jj + 1) * 512],
                            wih_s[:, (i % 8) * 3 * P + 2 * P:(i % 8) * 3 * P + 3 * P],
                            lat_ch[:, par * BT + jj * 512:par * BT + (jj + 1) * 512],
                            start=(i == 8), stop=(i == 15))
                    mm.then_inc(s_xpk, 1)
                # sweeps
                for s in range(1, NSWEEP + 1):
                    tensor.wait_ge(rsem, 16 * s)          # b0 halves arrived
                    tensor.wait_ge(s_pre, 2 * s)          # xp preloaded into psum
                    for half in range(2):
                        if half == 1:
                            tensor.wait_ge(rsem2, 16 * s)  # b1 halves arrived
                        for k in range(8):
                            for g in range(2):
                                for jj in (0, 1) if half == 0 else (2, 3):
                                    mm = tensor.matmul(
                                        (pa_r if g == 0 else pa_z)[:, jj * 512:(jj + 1) * 512],
                                        whh_s[:, k * 3 * P + g * P:k * 3 * P + (g + 1) * P],
                                        Hsh[:, hsh_base(s, k) + jj * 512:hsh_base(s, k) + (jj + 1) * 512],
                                        start=False, stop=(k == 7),
                                        skip_group_check=True)
                    mm.then_inc(s_mm, 1)                  # -> 2s-1
                    tensor.wait_ge(s_sig, 2 * s)          # sigma_r drained pa_r
                    for k in range(8):
                        for jj in range(4):
                            mm = tensor.matmul(
                                pa_r[:, jj * 512:(jj + 1) * 512],
                                whh_s[:, k * 3 * P + 2 * P:k * 3 * P + 3 * P],
                                Hsh[:, hsh_base(s, k) + jj * 512:hsh_base(s, k) + (jj + 1) * 512],
                                start=(k == 0), stop=(k == 7))
                    mm.then_inc(s_mm, 1)                  # -> 2s
                # beta partial
                tensor.wait_ge(s_h, NSWEEP + 1)
                tensor.wait_ge(s_h1, NSWEEP + 1)
                for jj, lo in enumerate((1, 513, 1041, 1553)):
                    tensor.matmul(pa_z[0:1, jj * 512:(jj + 1) * 512], bw_s[:],
                                  Hbf[:, lo:lo + 512],
                                  start=True, stop=True).then_inc(s_pb, 1)

            @block.scalar
            def _(scalar):
                # xp drains
                scalar.wait_ge(s_xpk, 8)
                scalar.activation(xp_s[:, 0:BT], pa_r[:], AF.Copy).then_inc(s_xpc, 1)
                scalar.activation(xp_s[:, BT:2 * BT], pa_z[:], AF.Copy).then_inc(s_xpc, 1)
                scalar.wait_ge(s_xpk, 16)
                scalar.activation(xp_s[:, 2 * BT:3 * BT], pa_r[:], AF.Copy).then_inc(s_xpc, 1)
                # init: z0 = sig(xp_z), n0 = tanh(xp_n)
                scalar.activation(rz_s[:, BT:2 * BT], xp_s[:, BT:2 * BT],
                                  AF.Sigmoid).then_inc(s_sig, 1)
                scalar.activation(tcn[:], xp_s[:, 2 * BT:3 * BT],
                                  AF.Tanh).then_inc(s_tanh, 1)
                for s in range(1, NSWEEP + 1):
                    if s >= 2:
                        scalar.wait_ge(s_tn2, s - 1)      # pa_r free (hn consumed)
                    scalar.activation(pa_r[:], xp_s[:, 0:BT], AF.Copy).then_inc(s_pre, 1)
                    scalar.activation(pa_z[:], xp_s[:, BT:2 * BT], AF.Copy).then_inc(s_pre, 1)
                    scalar.wait_ge(s_mm, 2 * s - 1)
                    scalar.wait_ge(s_h1, s)               # rz_s WAR
                    scalar.activation(rz_s[:, 0:BT], pa_r[:], AF.Sigmoid).then_inc(s_sig, 1)
                    scalar.activation(rz_s[:, BT:2 * BT], pa_z[:], AF.Sigmoid).then_inc(s_sig, 1)
                    scalar.wait_ge(s_tn, s)
                    scalar.activation(tcn[:, 0:N], tb[:, 0:N], AF.Tanh).then_inc(s_tanh, 1)
                    scalar.wait_ge(s_tn2, s)
                    scalar.activation(tcn[:, N:BT], tb[:, N:BT], AF.Tanh).then_inc(s_tanh, 1)
                scalar.wait_ge(s_pb, 4)
                scalar.activation(betap_s[:], pa_z[0:1, 0:BT], AF.Copy).then_inc(s_ab, 1)

            @block.vector
            def _(vector):
                vector.memset(Hbf[:, 0:1], 0.0)
                vector.memset(Hbf[:, 1040:1041], 0.0)
                # init: zn = z0*n0 ; bt = n0 - zn ; tts per batch
                vector.wait_ge(s_sig, 1)
                vector.wait_ge(s_tanh, 1)
                vector.tensor_mul(ta[:], rz_s[:, BT:2 * BT], tcn[:])
                vector.tensor_sub(tb[:], tcn[:], ta[:])
                vector.tensor_tensor_scan(Hbf[:, 1:N + 1], rz_s[:, BT:BT + N],
                                          tb[:, 0:N], 0.0, ALU.mult, ALU.add
                                          ).then_inc(s_h, 1)
                vector.tensor_tensor_scan(Hbf[:, 1041:1041 + N], rz_s[:, BT + N:2 * BT],
                                          tb[:, N:BT], 0.0, ALU.mult, ALU.add
                                          ).then_inc(s_h1, 1)
                for s in range(1, NSWEEP + 1):
                    # b0 chain first, fire its broadcast, then b1
                    vector.wait_ge(s_mm, 2 * s)
                    vector.wait_ge(s_sig, 2 * s)
                    vector.tensor_mul(ta[:, 0:N], rz_s[:, 0:N], pa_r[:, 0:N])
                    vector.tensor_add(tb[:, 0:N], ta[:, 0:N], xp_s[:, 2 * BT:2 * BT + N]
                                      ).then_inc(s_tn, 1)
                    vector.wait_ge(s_sig, 2 * s + 1)
                    vector.wait_ge(s_tanh, 2 * s)
                    vector.tensor_mul(ta[:, 0:N], rz_s[:, BT:BT + N], tcn[:, 0:N])
                    vector.tensor_sub(tb[:, 0:N], tcn[:, 0:N], ta[:, 0:N])
                    vector.wait_ge(lsem, 32 * s)          # Hbf WAR vs round s-1
                    vector.tensor_tensor_scan(Hbf[:, 1:N + 1], rz_s[:, BT:BT + N],
                                              tb[:, 0:N], 0.0, ALU.mult, ALU.add
                                              ).then_inc(s_h, 1)
                    vector.tensor_mul(ta[:, N:BT], rz_s[:, N:BT], pa_r[:, N:BT])
                    vector.tensor_add(tb[:, N:BT], ta[:, N:BT],
                                      xp_s[:, 2 * BT + N:3 * BT]).then_inc(s_tn2, 1)
                    vector.wait_ge(s_tanh, 2 * s + 1)
                    vector.tensor_mul(ta[:, N:BT], rz_s[:, BT + N:2 * BT], tcn[:, N:BT])
                    vector.tensor_sub(tb[:, N:BT], tcn[:, N:BT], ta[:, N:BT])
                    vector.tensor_tensor_scan(Hbf[:, 1041:1041 + N], rz_s[:, BT + N:2 * BT],
                                              tb[:, N:BT], 0.0, ALU.mult, ALU.add
                                              ).then_inc(s_h1, 1)

            @block.gpsimd
            def _(gpsimd):
                gpsimd.load_library(library_config.remote_dma)
                regs = []
                gpsimd.wait_ge(s_slot, 16)
                for j in range(4):
                    r = gpsimd.alloc_register(f"slot_r{j}")
                    gpsimd.reg_load(r, slot_s[0:1, j:j + 1])
                    regs.append(gpsimd.snap(r, donate=True, min_val=0,
                                            max_val=2 * 8 * BT - N))
                rdests = [(0, k) for k in range(8)]
                def prep_round(rnd):
                    par = rnd % 2
                    gpsimd.remote_dma_broadcast(
                        Hsh[:, ds(regs[2 * par + 0], N)], Hbf[:, 0:N],
                        rsem, lsem, rdests=rdests).then_inc(s_prep, 1)
                    gpsimd.remote_dma_broadcast(
                        Hsh[:, ds(regs[2 * par + 1], N)], Hbf[:, 1040:1040 + N],
                        rsem2, lsem, rdests=rdests).then_inc(s_prep, 1)
                if NSWEEP <= 8:
                    # pre-generate every round's descriptors (registers static)
                    for rnd in range(NSWEEP):
                        prep_round(rnd)
                    gpsimd.wait_ge(s_prep, 2 * NSWEEP)
                    for rnd in range(NSWEEP):
                        gpsimd.wait_ge(s_h, rnd + 1)
                        gpsimd.trigger_dma(1)
                        gpsimd.wait_ge(s_h1, rnd + 1)
                        gpsimd.trigger_dma(1)
                else:
                    # ring-capacity-safe: prep each round just in time
                    for rnd in range(NSWEEP):
                        prep_round(rnd)
                        gpsimd.wait_ge(s_prep, 2 * (rnd + 1))
                        gpsimd.wait_ge(s_h, rnd + 1)
                        gpsimd.trigger_dma(1)
                        gpsimd.wait_ge(s_h1, rnd + 1)
                        gpsimd.trigger_dma(1)
    return nc


def _p1_finish(results):
    tot = np.zeros((1, BT), np.float64)
    for c in range(8):
        tot += np.asarray(results[c]["betap"], np.float64)
    beta = 1.0 / (1.0 + np.exp(-tot.reshape(B, N)))
    return beta.astype(np.float32)


# ------------------------------------------------------------ P2 (scan+dec)
def _p2_host_prep(inputs, beta, core):
    lat = np.asarray(inputs["latent"], np.float32)
    dec_w1 = np.asarray(inputs["dec_w1"], np.float32)
    dec_b1 = np.asarray(inputs["dec_b1"], np.float32)
    dec_w2 = np.asarray(inputs["dec_w2"], np.float32)
    dec_b2 = np.asarray(inputs["dec_b2"], np.float32)
    c = core
    bf = ml_dtypes.bfloat16

    d_perm = np.concatenate([np.arange(c * P, (c + 1) * P),
                             np.delete(np.arange(D), np.arange(c * P, (c + 1) * P))])
    latTd = np.ascontiguousarray(lat.transpose(2, 0, 1).reshape(D, B * N)[d_perm], np.float32)
    bbc = np.ascontiguousarray(np.repeat(beta.reshape(1, B * N), P, axis=0), np.float32)
    rows = (c * P + np.arange(P)[None, :]) * R + np.arange(R)[:, None]
    w2T_shard = np.ascontiguousarray(dec_w2[rows.reshape(-1), :].T).astype(bf)
    b2w1 = np.ascontiguousarray(dec_b2[rows], np.float32)
    W2s = dec_w2[D * R:].reshape(D, R, H).sum(0)
    b2s = dec_b2[D * R:].reshape(D, R).sum(0)[:, None]
    return {
        "latTd": latTd[0:P],
        "latTd_bf": latTd[P:].astype(bf),
        "bbc": bbc,
        "w1T": np.ascontiguousarray(dec_w1[:, d_perm].T).astype(bf),
        "b1_pc": np.ascontiguousarray(dec_b1.reshape(16, P).T, np.float32),
        "W2sT": np.ascontiguousarray(W2s.T).astype(bf),
        "b2s_pc": np.ascontiguousarray(b2s, np.float32),
        "w2T_shard": w2T_shard,
        "b2w1": b2w1,
    }


def _p2_build(nc):
    from contextlib import ExitStack
    latTd = nc.declare_dram_parameter("latTd", [P, B * N], F32, isOutput=False)
    latTd_bf = nc.declare_dram_parameter("latTd_bf", [D - P, B * N], BF16, isOutput=False)
    bbc = nc.declare_dram_parameter("bbc", [P, B * N], F32, isOutput=False)
    w1T = nc.declare_dram_parameter("w1T", [D, H], BF16, isOutput=False)
    b1_pc = nc.declare_dram_parameter("b1_pc", [P, 16], F32, isOutput=False)
    W2sT = nc.declare_dram_parameter("W2sT", [H, R], BF16, isOutput=False)
    b2s_pc = nc.declare_dram_parameter("b2s_pc", [R, 1], F32, isOutput=False)
    w2T_shard = nc.declare_dram_parameter("w2T_shard", [H, H], BF16, isOutput=False)
    b2w1 = nc.declare_dram_parameter("b2w1", [R, P], F32, isOutput=False)
    outT = nc.declare_dram_parameter("outT", [P, B * N], F32, isOutput=True)
    w2s_dram = nc.dram_tensor("w2s_dram", [R, B * N], F32)

    with TileContext(nc) as tc, ExitStack() as ctx:
        const = ctx.enter_context(tc.tile_pool(name="const", bufs=1))
        persist = ctx.enter_context(tc.tile_pool(name="persist", bufs=1))
        lhs_pool = ctx.enter_context(tc.tile_pool(name="lhs", bufs=4))
        work = ctx.enter_context(tc.tile_pool(name="work", bufs=3))
        pbig = ctx.enter_context(tc.tile_pool(name="pbig", bufs=2, space="PSUM"))
        psmall = ctx.enter_context(tc.tile_pool(name="psmall", bufs=2, space="PSUM"))

        b1t = const.tile([P, 16], F32, tag="b1t")
        nc.sync.dma_start(out=b1t[:], in_=b1_pc[:, :])
        b2st = const.tile([R, 1], F32, tag="b2st")
        nc.sync.dma_start(out=b2st[:], in_=b2s_pc[:, :])
        b2w1t = const.tile([R, P], F32, tag="b2w1t")
        nc.sync.dma_start(out=b2w1t[:], in_=b2w1[:, :])
        latTt = const.tile([P, B * N], F32, tag="latTt")
        nc.sync.dma_start(out=latTt[:], in_=latTd[:, :])
        bbct = const.tile([P, B * N], F32, tag="bbct")
        nc.sync.dma_start(out=bbct[:], in_=bbc[:, :])

        gT = [[persist.tile([P, N], BF16, tag=f"g{b}_{dm}", name=f"g{b}_{dm}") for dm in range(8)]
              for b in range(B)]
        gown = persist.tile([P, B * N], F32, tag="gown")
        hid = [persist.tile([P, B * N], BF16, tag=f"hid{m}", name=f"hid{m}") for m in range(16)]
        w2st = persist.tile([R, B * N], F32, tag="w2st")
        acc = persist.tile([P, B * N], F32, tag="acc")

        # Phase 1: gated scan (dm=0 from resident fp32 latTt; rest bf16)
        for dm in range(8):
            if dm > 0:
                ldt = work.tile([P, B * N], BF16, tag="ldt", bufs=2, name="ldt")
                nc.sync.dma_start(out=ldt[:], in_=latTd_bf[(dm - 1) * P:dm * P, :])
            for b in range(B):
                sl = slice(b * N, (b + 1) * N)
                if dm == 0:
                    nc.vector.tensor_tensor_scan(gown[:, sl], bbct[:, sl], latTt[:, sl],
                                                 0.0, mybir.AluOpType.mult,
                                                 mybir.AluOpType.add)
                    nc.scalar.activation(gT[b][0][:, :], gown[:, sl], AF.Copy)
                else:
                    nc.vector.tensor_tensor_scan(gT[b][dm][:, :], bbct[:, sl], ldt[:, sl],
                                                 0.0, mybir.AluOpType.mult,
                                                 mybir.AluOpType.add)

        # Phase 2: mm1 -> hid (native tanh-approx gelu); one lhsT DMA per m
        for m in range(16):
            wt = lhs_pool.tile([P, 8 * P], BF16, tag="w1lhs", name="w1lhs")
            nc.sync.dma_start(out=wt[:].rearrange("p (k m) -> p k m", k=8),
                              in_=w1T[:, m * P:(m + 1) * P]
                              .rearrange("(k p) m -> p k m", p=P))
            for b in range(B):
                ph = pbig.tile([P, N], F32, tag="big", name="ph")
                for k in range(8):
                    for jj in range(2):
                        nc.tensor.matmul(ph[:, jj * 512:(jj + 1) * 512],
                                         wt[:, k * P:(k + 1) * P],
                                         gT[b][k][:, jj * 512:(jj + 1) * 512],
                                         start=(k == 0), stop=(k == 7))
                nc.scalar.activation(hid[m][:, b * N:(b + 1) * N], ph[:],
                                     AF.Gelu_apprx_tanh, bias=b1t[:, m:m + 1])

        # Phase 3: w2s (single lhsT DMA for all 16 chunks)
        w2sw = const.tile([P, 16 * R], BF16, tag="w2slhs")
        nc.sync.dma_start(out=w2sw[:].rearrange("p (k r) -> p k r", k=16),
                          in_=W2sT[:, :].rearrange("(k p) r -> p k r", p=P))
        for n in range(2):
            pw = pbig.tile([R, N], F32, tag="big", name="pw")
            for k in range(16):
                for jj in range(2):
                    nc.tensor.matmul(pw[:, jj * 512:(jj + 1) * 512],
                                     w2sw[:, k * R:(k + 1) * R],
                                     hid[k][:, n * N + jj * 512:n * N + (jj + 1) * 512],
                                     start=(k == 0), stop=(k == 15))
            nc.scalar.activation(w2st[:, n * N:(n + 1) * N], pw[:], AF.Identity,
                                 bias=b2st[:, 0:1])
            nc.sync.dma_start(out=w2s_dram[:, n * N:(n + 1) * N], in_=w2st[:, n * N:(n + 1) * N])

        # Phase 4: acc seed + mm2 + r-contraction
        for n in range(4):
            psd = psmall.tile([P, 512], F32, tag="small", name="psd")
            nc.tensor.matmul(psd[:], b2w1t[:], w2st[:, n * 512:(n + 1) * 512],
                             start=True, stop=True)
            nc.scalar.activation(acc[:, n * 512:(n + 1) * 512], psd[:], AF.Copy)

        for m in range(16):
            wt2 = lhs_pool.tile([P, 16 * P], BF16, tag="w2lhs", name="w2lhs")
            nc.sync.dma_start(out=wt2[:].rearrange("p (k m) -> p k m", k=16),
                              in_=w2T_shard[:, m * P:(m + 1) * P]
                              .rearrange("(k p) m -> p k m", p=P))
            for n in range(2):
                pm = pbig.tile([P, N], F32, tag="big", name="pm")
                for k in range(16):
                    for jj in range(2):
                        nc.tensor.matmul(pm[:, jj * 512:(jj + 1) * 512],
                                         wt2[:, k * P:(k + 1) * P],
                                         hid[k][:, n * N + jj * 512:n * N + (jj + 1) * 512],
                                         start=(k == 0), stop=(k == 15))
                wb = work.tile([P, N], F32, tag="tmpA", bufs=2, name="wb")
                nc.sync.dma_start(out=wb[:], in_=w2s_dram[m:m + 1, n * N:(n + 1) * N]
                                  .to_broadcast([P, N]))
                tmp = work.tile([P, N], F32, tag="tmpB", bufs=2, name="tmp")
                nc.vector.tensor_mul(tmp[:], pm[:], wb[:])
                nc.vector.tensor_add(acc[:, n * N:(n + 1) * N],
                                     acc[:, n * N:(n + 1) * N], tmp[:])

        # Phase 5: out = latT + gown * acc
        for n in range(2):
            sl = slice(n * N, (n + 1) * N)
            ctrl = work.tile([P, N], F32, tag="tmpA", bufs=2, name="ctrl")
            nc.vector.tensor_mul(ctrl[:], acc[:, sl], gown[:, sl])
            ot = work.tile([P, N], F32, tag="tmpB", bufs=2, name="ot")
            nc.vector.tensor_add(ot[:], ctrl[:], latTt[:, sl])
            nc.sync.dma_start(out=outT[:, sl], in_=ot[:])
    return nc


def _p2_finish(results):
    out = np.empty((B, N, D), np.float32)
    for c in range(8):
        o = np.asarray(results[c]["outT"])
        out[:, :, c * P:(c + 1) * P] = o.reshape(P, B, N).transpose(1, 2, 0)
    return out


# ----------------------------------------------------------------- kernel()
_cache = {}
last_results = {}


def _get_programs():
    if "nc1" not in _cache:
        nc1 = bass.Bass()
        _p1_build(nc1)
        library_overlay.lower_extended_insts(nc1)
        _cache["nc1"] = nc1
        nc2 = bacc.Bacc(None, target_bir_lowering=False)
        _p2_build(nc2)
        nc2.finalize()
        _cache["nc2"] = nc2
    return _cache["nc1"], _cache["nc2"]


def kernel(**inputs):
    import os
    tdir = os.environ.get("BASS_KERNEL_TRACE")
    kw1, kw2 = {}, {}
    if tdir:
        os.makedirs(f"{tdir}/p1", exist_ok=True)
        os.makedirs(f"{tdir}/p2", exist_ok=True)
        kw1 = dict(trace=True, tmpdir=f"{tdir}/p1")
        kw2 = dict(trace=True, tmpdir=f"{tdir}/p2")
    nc1, nc2 = _get_programs()
    maps1 = [_p1_host_prep(inputs, c) for c in range(8)]
    r1 = run_bass_kernel_spmd(nc1, maps1, list(range(8)), **kw1)
    beta = _p1_finish(r1.results)
    maps2 = [_p2_host_prep(inputs, beta, c) for c in range(8)]
    r2 = run_bass_kernel_spmd(nc2, maps2, list(range(8)), **kw2)
    if tdir:
        last_results["p1"] = r1
        last_results["p2"] = r2
    return _p2_finish(r2.results)
